# revision 12
# baseline (speedup 1.0000x reference)
"""AxialAttention2D kernel for 8 TRN2 NeuronCores — v3.

Sharding: data-parallel over B (B == 8 == n_cores). Each core processes one
full [C, H, W] image. No collectives.

v3 structure:
- Attention output computed TRANSPOSED: avT[i,(h,d)] = matmul(lhsT=eT_slice,
  rhs=[vT_h | ones]) with N=33 — the ones column makes the softmax row-sum a
  free by-product (col 32), killing the separate ones-matmul rowsums and
  shrinking the reciprocal to 16 cols. A PE transpose restores [(h,d), i]
  for the projection.
- No fp32 accumulator: phase-0 h-attention for the first S_ITEMS rows
  (overlapping the input stream) stores `on_h_s`; phase-1 w-attention stores
  w-major `on_w`; phase-2 h-blocks run proj_h + proj_w accumulating in one
  PSUM tile -> single bias-activation -> contiguous fp32 out block -> DMA.
- Global software pipeline: SG i+1's qk/vT matmuls+copies are emitted
  between SG i's first score block and its AV stage so the PE always has
  dependency-free work during exp stalls (p-state stays hot).
- Scores in [C,1024] head-pair blocks (each head row-tile owns a PSUM bank).

Self-contained: shapes hardcoded (B=8, C=128, H=W=128, heads=4).
"""

import numpy as np
from contextlib import ExitStack

C = 128          # channels (= SBUF partitions)
L = 128          # attention sequence length (H or W)
HW = L * L       # flattened spatial size
HEADS = 4
HD = C // HEADS  # 32
SCALE = HD ** -0.5
SG = 4           # items per super-group
SL = SG * L      # 512
NG = SG * HEADS  # 16 (it,h) groups per super-group
GW = HD + 1      # 33: av cols + rowsum col per group
S_ITEMS = 40     # phase-0 H-items (overlap input DMA); multiple of SG
NCHUNK = 16
CHW = HW // NCHUNK  # 1024 columns per input chunk (8 rows)

_cache = {}

W_NAMES = ("wqT_h", "wkT_h", "wvT_h", "wpT_h", "wqT_w", "wkT_w", "wvT_w", "wpT_w")


def _build_nc():
    import concourse.bacc as bacc
    import concourse.tile as tile
    from concourse import mybir
    from concourse.alu_op_type import AluOpType as AluOp

    f32 = mybir.dt.float32
    bf16 = mybir.dt.bfloat16
    Exp = mybir.ActivationFunctionType.Exp
    Ident = mybir.ActivationFunctionType.Identity

    nc = bacc.Bacc(None, name="axial_attn")

    x_d = nc.dram_tensor("x", [C, HW], bf16, kind="ExternalInput")
    w_d = {n: nc.dram_tensor(n, [C, C], bf16, kind="ExternalInput") for n in W_NAMES}
    bias_d = nc.dram_tensor("bias", [C, 1], f32, kind="ExternalInput")
    ident_d = nc.dram_tensor("ident", [C, C], bf16, kind="ExternalInput")
    out_d = nc.dram_tensor("out", [C, HW], bf16, kind="ExternalOutput")

    with ExitStack() as ctx:
        tc = ctx.enter_context(tile.TileContext(nc))
        singles = ctx.enter_context(tc.tile_pool(name="singles", bufs=1))
        big = ctx.enter_context(tc.tile_pool(name="big", bufs=1))
        nrm = ctx.enter_context(tc.tile_pool(name="nrm", bufs=2))
        work = ctx.enter_context(tc.tile_pool(name="work", bufs=3))
        work2 = ctx.enter_context(tc.tile_pool(name="work2", bufs=2))
        # PSUM (KB/partition): s 2x4 double-buffered + shared ring 2x4 = 16.
        # The ring time-shares {vt, qk, avt, on_ps, p}; at most two are live
        # at once (vt+qk during an SG front).
        ps_s = ctx.enter_context(tc.tile_pool(name="ps_s", bufs=2, space="PSUM"))
        ps_ring = ctx.enter_context(tc.tile_pool(name="ps_ring", bufs=2, space="PSUM"))

        w_sb = {}
        for n in W_NAMES:
            w_sb[n] = singles.tile([C, C], bf16, tag=n, name=n)
            nc.sync.dma_start(out=w_sb[n][:], in_=w_d[n][:])
        bias_sb = singles.tile([C, 1], f32, tag="bias")
        nc.sync.dma_start(out=bias_sb[:], in_=bias_d[:])
        ident_sb = singles.tile([C, C], bf16, tag="ident")
        nc.sync.dma_start(out=ident_sb[:], in_=ident_d[:])

        xc = big.tile([C, HW], bf16, tag="x_bf16")       # 4 MB
        on_w = big.tile([C, HW], bf16, tag="on_w")       # 4 MB, (w,h) layout
        on_h_s = big.tile([C, S_ITEMS * L], bf16, tag="on_h_s")

        xc_v = xc[:].rearrange("c (h w) -> c w h", w=L)
        on_w_v = on_w[:].rearrange("c (w h) -> c h w", h=L)

        def load_chunk(ci, eng=None):
            sl = slice(ci * CHW, (ci + 1) * CHW)
            nc.sync.dma_start(out=xc[:, sl], in_=x_d[:, sl])

        def sg_front(passc, g0):
            """qkv matmuls + PSUM->SBUF copies for one super-group."""
            wq, wk, wv = (w_sb[f"w{t}T_{passc}"] for t in ("q", "k", "v"))

            def xs_item(it):
                if passc == "h":
                    return xc[:, (g0 + it) * L:(g0 + it + 1) * L]
                return xc_v[:, g0 + it, :]

            if passc == "h":
                rhs_qk = xc[:, g0 * L:(g0 + SG) * L]
            else:
                rhs_qk = xc_v[:, g0:g0 + SG, :]

            # vT matmuls first: if the PE is cold after an exp stall, pay the
            # low-p-state tax on N=128 streams, not the N=512 q/k streams.
            vt = ps_ring.tile([C, SL], f32, tag="ring", name="vt_ps")
            for it in range(SG):
                nc.tensor.matmul(vt[:, it * L:(it + 1) * L], xs_item(it), wv[:],
                                 start=True, stop=True)
            qk = ps_ring.tile([C, 2 * SL], f32, tag="ring", name="qk_ps")
            nc.tensor.matmul(qk[:, 0:SL], wq[:], rhs_qk, start=True, stop=True)
            nc.tensor.matmul(qk[:, SL:2 * SL], wk[:], rhs_qk, start=True, stop=True)
            qkv = work.tile([C, 2 * SL], bf16, tag="qkv", name="qkv_sb")
            nc.vector.tensor_copy(out=qkv[:], in_=qk[:])   # q|k, one 1024-col op
            # vte: per (it,h) group: [vT_h(it) (32) | 1.0] -> rhs of the AVT mm
            vte = work.tile([C, NG * GW], bf16, tag="vte", name="vte_sb")
            vte_g = vte[:].rearrange("c (g d) -> c g d", d=GW)
            nc.gpsimd.memset(vte_g[:, :, HD:GW], 1.0)
            nc.scalar.copy(out=vte_g[:, :, 0:HD],
                           in_=vt[:].rearrange("c (g d) -> c g d", d=HD))
            return qkv, vte

        def sg_back(passc, g0, fr, store_ap, fill=None, fill2=None):
            """Scores/exp/AV-T/normalize/transpose; writes [C, SL] bf16
            normalized per-head output (cols (item, seqpos)) to store_ap."""
            qkv, vte = fr
            eT = work2.tile([C, 2048], bf16, tag="eT", name="eT_sb")
            s_pair = [ps_s.tile([C, 1024], f32, tag="s", name="s_ps"),
                      ps_s.tile([C, 1024], f32, tag="s", name="s2_ps")]

            def scores(hpair):
                s = s_pair[hpair]
                for it in range(SG):
                    qoff = it * L
                    koff = SL + it * L
                    for hl in range(2):
                        h = 2 * hpair + hl
                        nc.tensor.matmul(
                            s[:, hl * 512 + it * L:hl * 512 + (it + 1) * L],
                            qkv[HD * h:HD * h + HD, koff:koff + L],
                            qkv[HD * h:HD * h + HD, qoff:qoff + L],
                            start=True, stop=True, tile_position=(HD * h, 0))

            # avt: two 512-col PSUM banks, 8 groups of 33 per bank so no
            # matmul output crosses a bank boundary.
            avt = ps_ring.tile([C, 1024], f32, tag="ring", name="avt_ps")

            def gcol(g):
                return (g // 8) * 512 + (g % 8) * GW

            def avts(hpair):
                for it in range(SG):
                    for hl in range(2):
                        h = 2 * hpair + hl
                        g = it * HEADS + h
                        nc.tensor.matmul(
                            avt[:, gcol(g):gcol(g) + GW],
                            eT[:, h * 512 + it * L:h * 512 + (it + 1) * L],
                            vte[:, g * GW:(g + 1) * GW],
                            start=True, stop=True)

            scores(0)
            scores(1)
            # previous SG's transposes/store/proj: PE runs them while this
            # SG's exp is on Scalar; their DVE deps (mul) finished during
            # the score matmuls above.
            if fill2 is not None:
                fill2()
            nc.scalar.activation(out=eT[:, 0:1024], in_=s_pair[0][:],
                                 func=Exp, scale=SCALE)
            nxt = fill() if fill is not None else None
            nc.scalar.activation(out=eT[:, 1024:2048], in_=s_pair[1][:],
                                 func=Exp, scale=SCALE)
            avts(0)
            avts(1)

            avt_j = avt[:].rearrange("c (p z) -> c p z", p=2)[:, :, 0:8 * GW] \
                .rearrange("c p (j d) -> c p j d", d=GW)
            rr = nrm.tile([C, NG], f32, tag="rr", name="rr_sb")
            nc.vector.reciprocal_approx_fast(
                out=rr[:].rearrange("c (p j) -> c p j", p=2),
                in_=avt_j[:, :, :, HD:GW].rearrange("c p j o -> c p (j o)"))
            onT = work2.tile([C, SL], bf16, tag="onT", name="onT_sb")
            rr_b = rr[:].rearrange("c (p j o) -> c p j o", p=2, o=1) \
                .broadcast_to([C, 2, 8, HD])
            nc.vector.tensor_mul(
                out=onT[:].rearrange("c (p j d) -> c p j d", p=2, d=HD),
                in0=avt_j[:, :, :, 0:HD], in1=rr_b)
            return nxt, onT

        def sg_tail(onT, store_ap):
            on_ps = ps_ring.tile([C, SL], bf16, tag="ring", name="on_ps")
            for it in range(SG):
                nc.tensor.transpose(on_ps[:, it * L:(it + 1) * L],
                                    onT[:, it * L:(it + 1) * L], ident_sb[:])
            nc.vector.tensor_copy(out=store_ap, in_=on_ps[:])

        def proj_sg(g0, on_src):
            """proj_h(on_src) + proj_w(on_w slice) + bias -> out block."""
            p = ps_ring.tile([C, SL], f32, tag="ring", name="p_ps")
            nc.tensor.matmul(p[:], w_sb["wpT_h"][:], on_src,
                             start=True, stop=False)
            nc.tensor.matmul(p[:], w_sb["wpT_w"][:], on_w_v[:, g0:g0 + SG, :],
                             start=False, stop=True)
            outb = work2.tile([C, SL], bf16, tag="outb", name="out_sb")
            nc.scalar.activation(out=outb[:], in_=p[:], func=Ident,
                                 bias=bias_sb[:], scale=1.0)
            nc.sync.dma_start(out=out_d[:, g0 * L:(g0 + SG) * L], in_=outb[:])

        # ---- global pipelined schedule ----
        n_s_chunks = S_ITEMS * L // CHW  # 5
        for ci in range(n_s_chunks):
            load_chunk(ci)

        # SG descriptors: (passc, g0, store_ap, proj_g0 | None)
        sgs = []
        for g0 in range(0, S_ITEMS, SG):
            sgs.append(("h", g0, on_h_s[:, g0 * L:(g0 + SG) * L], None))
        for g0 in range(0, L, SG):
            sgs.append(("w", g0, on_w[:, g0 * L:(g0 + SG) * L], None))
        fresh = list(range(S_ITEMS, L, SG))
        for g0 in fresh:
            sgs.append(("h", g0, None, g0))  # store to rotating onb, then proj

        late_chunks = list(range(n_s_chunks, NCHUNK))
        proj_only = list(range(0, S_ITEMS, SG))
        n_w0 = len([1 for s_ in sgs if s_[0] == "h" and s_[3] is None])  # 10

        fr = sg_front(sgs[0][0], sgs[0][1])
        pend = [None]  # deferred tail (transpose/store/proj) of previous SG
        for i, (passc, g0, store_ap, pj) in enumerate(sgs):
            if store_ap is None:
                store_ap = work2.tile([C, SL], bf16, tag="onb", name="onb_sb")[:]

            def fill(i=i, pj=pj):
                # All chunks must be cast before the first w-pass front
                # (emitted at i == n_w0 - 1) is issued.
                if late_chunks:
                    n_load = len(late_chunks) if i >= n_w0 - 1 else 1
                    for _ in range(n_load):
                        ci = late_chunks.pop(0)
                        load_chunk(ci)
                nxt = None
                if i + 1 < len(sgs):
                    nxt = sg_front(sgs[i + 1][0], sgs[i + 1][1])
                # sprinkle proj-only groups through phase-2 as PE filler
                if pj is not None and proj_only:
                    g1 = proj_only.pop(0)
                    proj_sg(g1, on_h_s[:, g1 * L:(g1 + SG) * L])
                return nxt

            def fill2(pend=pend):
                if pend[0] is not None:
                    pend[0]()
                    pend[0] = None

            fr, onT = sg_back(passc, g0, fr, store_ap, fill=fill, fill2=fill2)

            def tail(onT=onT, store_ap=store_ap, pj=pj):
                sg_tail(onT, store_ap)
                if pj is not None:
                    proj_sg(pj, store_ap)

            pend[0] = tail
        if pend[0] is not None:
            pend[0]()
        for ci in late_chunks:
            load_chunk(ci)
        for g1 in proj_only:
            proj_sg(g1, on_h_s[:, g1 * L:(g1 + SG) * L])

    nc.finalize()
    return nc


def _get_nc():
    if "nc" not in _cache:
        _cache["nc"] = _build_nc()
    return _cache["nc"]


def _make_in_maps(x, wqkv_h, wproj_h, bproj_h, wqkv_w, wproj_w, bproj_w):
    import ml_dtypes
    bf = ml_dtypes.bfloat16
    x = np.asarray(x, dtype=np.float32)
    xb = x.astype(bf)  # wire format: bf16 (kernel consumed x as bf16 anyway)
    B = x.shape[0]

    def wT(w):
        return np.ascontiguousarray(np.asarray(w, np.float32).T)

    common = {
        "wqT_h": wT(wqkv_h[0:C]).astype(bf),
        "wkT_h": wT(wqkv_h[C:2 * C]).astype(bf),
        "wvT_h": wT(wqkv_h[2 * C:3 * C]).astype(bf),
        "wpT_h": (wT(wproj_h) * 0.5).astype(bf),
        "wqT_w": wT(wqkv_w[0:C]).astype(bf),
        "wkT_w": wT(wqkv_w[C:2 * C]).astype(bf),
        "wvT_w": wT(wqkv_w[2 * C:3 * C]).astype(bf),
        "wpT_w": (wT(wproj_w) * 0.5).astype(bf),
        "bias": (0.5 * (np.asarray(bproj_h, np.float32)
                        + np.asarray(bproj_w, np.float32))).reshape(C, 1),
        "ident": np.eye(C, dtype=np.float32).astype(bf),
    }
    return [
        {**common, "x": xb[b].reshape(C, HW)}
        for b in range(B)
    ]


def _run(in_maps, **kw):
    from concourse.bass_utils import run_bass_kernel_spmd
    nc = _get_nc()
    res = run_bass_kernel_spmd(nc, in_maps, core_ids=list(range(len(in_maps))), **kw)
    _cache["last_results"] = res
    return res


def kernel(x, wqkv_h, wproj_h, bproj_h, wqkv_w, wproj_w, bproj_w):
    in_maps = _make_in_maps(x, wqkv_h, wproj_h, bproj_h,
                            wqkv_w, wproj_w, bproj_w)
    res = _run(in_maps)
    out = np.stack([r["out"].reshape(C, L, L) for r in res.results], axis=0)
    return out.astype(np.float32)  # wire is bf16; upcast for the caller

